# revision 5
# baseline (speedup 1.0000x reference)
"""Trainium2 Bass kernel v4 for nn_AttentionLayer.

Math (per core, vocab-sharded): out[b, v'] = occ[b, v'] * leaky_relu(t[v'] + s[b])
with t = table_shard^T a_w (PE, bf16), s = attr_emb @ a_a (DVE, f32).

v4 vs v3 (45.4us):
  - occ mask int8 (1.64 MB vs 3.28 bf16): DVE TT drops to 1x mode
    (+0.8us/tile) but HBM traffic/core falls to 8.2 MB (floor ~23us).
  - all DRAM tensors partition-major on host: every DMA is 128 fully
    contiguous descriptors, no rearrange fan-out.
  - inputs land as 4 table chunks (dh x vocab-group) + 2 occ chunks,
    ordered so strip 0 matmul starts after ~4us.
  - outputs grouped into 4 big DMAs issued on the scalar (qActDynamicHW)
    ring so stores never queue behind loads on the sync ring.
"""

import numpy as np
import ml_dtypes

import concourse.bass as bass
import concourse.tile as tile
from concourse import bacc, mybir
from concourse.bass_utils import run_bass_kernel_spmd

B = 256
L = 512
V = 50257
DW = 256
DA = 256
ALPHA = 0.2

NCORES = 8
VS = 6400          # vocab span per core
SW = 1280          # strip width
NS = VS // SW      # 5 strips
GA = 2 * SW        # vocab-group A: strips 0-1  -> cols [0, 2560)
GB = VS - GA       # vocab-group B: strips 2-4  -> cols [2560, 6400)

BF16 = ml_dtypes.bfloat16

_CACHE = {}


def _build():
    if "nc" in _CACHE:
        return _CACHE["nc"]
    f32 = mybir.dt.float32
    bf16 = mybir.dt.bfloat16
    i8 = mybir.dt.int8

    nc = bacc.Bacc("TRN2", target_bir_lowering=False, debug=False)
    tbl = nc.declare_dram_parameter("tbl", [128, 2 * VS], bf16, isOutput=False)
    occ = nc.declare_dram_parameter("occ", [128, 2 * VS], i8, isOutput=False)
    awb = nc.declare_dram_parameter("awb", [128, 2 * 128], bf16, isOutput=False)
    aa = nc.declare_dram_parameter("aa", [128, DA], f32, isOutput=False)
    attr = nc.declare_dram_parameter("attr", [128, 2 * DA], f32, isOutput=False)
    out = nc.declare_dram_parameter("out", [128, 2 * VS], bf16, isOutput=True)

    with tile.TileContext(nc) as tc:
        with (
            tc.tile_pool(name="sb", bufs=1) as sb,
            tc.tile_pool(name="pst", bufs=2, space="PSUM") as pst,
        ):
            awb_t = sb.tile([128, 2 * 128], bf16, tag="awb")
            nc.sync.dma_start(awb_t[:], awb.ap())
            aa_t = sb.tile([128, DA], f32, tag="aa")
            nc.sync.dma_start(aa_t[:], aa.ap())
            at = sb.tile([128, 2 * DA], f32, tag="attr")
            nc.sync.dma_start(at[:], attr.ap())

            # ---- s = attr_emb @ a_a  (s_sb[:, h] holds b = h*128 + p) ----
            s_sb = sb.tile([128, 2], f32, tag="s")
            for h in range(2):
                pa = sb.tile([128, DA], f32, tag=f"pa{h}")
                nc.vector.tensor_tensor(
                    out=pa[:],
                    in0=at[:, h * DA : (h + 1) * DA],
                    in1=aa_t[:],
                    op=mybir.AluOpType.mult,
                )
                nc.vector.tensor_reduce(
                    out=s_sb[:, h : h + 1],
                    in_=pa[:],
                    axis=mybir.AxisListType.X,
                    op=mybir.AluOpType.add,
                )

            # ---- input chunks, in pipeline order ----
            # table: (dh, group) chunks; group A first so strip 0 starts early
            tA = [sb.tile([128, GA], bf16, tag=f"tA{dh}", name=f"tA{dh}") for dh in range(2)]
            tB = [sb.tile([128, GB], bf16, tag=f"tB{dh}", name=f"tB{dh}") for dh in range(2)]
            for dh in range(2):
                nc.sync.dma_start(tA[dh][:], tbl.ap()[:, dh * VS : dh * VS + GA])
            for dh in range(2):
                nc.sync.dma_start(tB[dh][:], tbl.ap()[:, dh * VS + GA : (dh + 1) * VS])
            oc = [sb.tile([128, VS], i8, tag=f"oc{h}", name=f"oc{h}") for h in range(2)]
            for h in range(2):
                nc.sync.dma_start(oc[h][:], occ.ap()[:, h * VS : (h + 1) * VS])

            # ---- output group tiles [h][group] ----
            og = [
                [
                    sb.tile([128, GA], bf16, tag=f"ogA{h}", name=f"ogA{h}"),
                    sb.tile([128, GB], bf16, tag=f"ogB{h}", name=f"ogB{h}"),
                ]
                for h in range(2)
            ]

            # ---- per strip ----
            o1s = []
            for si in range(NS):
                grp = 0 if si < 2 else 1
                tt = tA if grp == 0 else tB
                base = 0 if grp == 0 else GA
                off = si * SW - base
                pt = pst.tile([128, SW], f32, tag="pt")
                for dh in range(2):
                    for n0, n1 in ((0, 512), (512, 1024), (1024, SW)):
                        nc.tensor.matmul(
                            pt[:, n0:n1],
                            lhsT=awb_t[:, dh * 128 : (dh + 1) * 128],
                            rhs=tt[dh][:, off + n0 : off + n1],
                            start=(dh == 0),
                            stop=(dh == 1),
                        )
                # both ACT passes drain pt immediately -> PSUM recycles fast
                for h in range(2):
                    o1 = sb.tile([128, SW], bf16, tag=f"o1_{si}_{h}", name=f"o1_{si}_{h}")
                    nc.scalar.activation(
                        o1[:],
                        pt[:],
                        mybir.ActivationFunctionType.Prelu,
                        bias=s_sb[:, h : h + 1],
                        scale=1.0,
                        alpha=ALPHA,
                    )
                    o1s.append((si, h, grp, off, o1))

            # ---- mask + store (TT gated on occ chunk arrival) ----
            for si, h, grp, off, o1 in sorted(o1s, key=lambda r: (r[1], r[0])):
                nc.vector.tensor_tensor(
                    out=og[h][grp][:, off : off + SW],
                    in0=o1[:],
                    in1=oc[h][:, si * SW : (si + 1) * SW],
                    op=mybir.AluOpType.mult,
                )
            for h in range(2):
                nc.scalar.dma_start(
                    out.ap()[:, h * VS : h * VS + GA], og[h][0][:]
                )
                nc.scalar.dma_start(
                    out.ap()[:, h * VS + GA : (h + 1) * VS], og[h][1][:]
                )

    nc.compile()
    _CACHE["nc"] = nc
    return nc


def _pmaj(x):
    """[256, N] -> partition-major [128, 2*N] (halves along columns)."""
    n = x.shape[1]
    return np.ascontiguousarray(
        x.reshape(2, 128, n).transpose(1, 0, 2).reshape(128, 2 * n)
    )


def _prep_inputs(words, word_emb_table, attr_emb, a):
    words = np.ascontiguousarray(words).astype(np.int64)
    wet = np.ascontiguousarray(word_emb_table, dtype=np.float32)
    attr = np.ascontiguousarray(attr_emb, dtype=np.float32)
    a = np.ascontiguousarray(a, dtype=np.float32).reshape(-1)

    # awb_dev[p, dh*128 + m] = a[dh*128 + p]
    A = a[:DW].astype(BF16).reshape(2, 128)
    awb_dev = np.ascontiguousarray(
        np.repeat(A.T[:, :, None], 128, axis=2).reshape(128, 2 * 128)
    )
    aa_rep = np.ascontiguousarray(np.broadcast_to(a[DW:][None, :], (128, DA)))
    attr_dev = _pmaj(attr)

    tblpad = np.zeros((NCORES * VS, DW), dtype=np.float32)
    tblpad[:V] = wet
    tbl_bf = tblpad.astype(BF16)

    occ_full = np.zeros((B, NCORES * VS), dtype=np.int8)
    rows = np.repeat(np.arange(B), L)
    occ_full[rows, words.reshape(-1)] = 1

    in_maps = []
    for i in range(NCORES):
        blk = tbl_bf[i * VS : (i + 1) * VS, :]          # [VS, 256]
        tbl_dev = _pmaj(np.ascontiguousarray(blk.T))    # [128, 2*VS]
        occ_dev = _pmaj(occ_full[:, i * VS : (i + 1) * VS])
        in_maps.append(
            {
                "tbl": tbl_dev,
                "occ": occ_dev,
                "awb": awb_dev,
                "aa": aa_rep,
                "attr": attr_dev,
            }
        )
    return in_maps


def kernel(words, word_emb_table, attr_emb, a, _trace=False, **_kw):
    nc = _build()
    in_maps = _prep_inputs(words, word_emb_table, attr_emb, a)
    res = run_bass_kernel_spmd(nc, in_maps, list(range(NCORES)), trace=_trace)
    parts = []
    for i in range(NCORES):
        o = res.results[i]["out"]                       # [128, 2*VS] bf16
        parts.append(o.reshape(128, 2, VS).transpose(1, 0, 2).reshape(B, VS))
    out = np.ascontiguousarray(
        np.concatenate(parts, axis=1)[:, :V].astype(np.float32)
    )
    if _trace:
        return out, res
    return out


# revision 6
# speedup vs baseline: 1.0976x; 1.0976x over previous
"""Trainium2 Bass kernel v5 for nn_AttentionLayer.

Math (per core, vocab-sharded): out[b, v'] = occ[b, v'] * leaky_relu(t[v'] + s[b])
with t = table_shard^T a_w (PE, bf16), s = attr_emb @ a_a (DVE, f32).

v5 vs v4 (46.4us): v4's single sync-ring FIFO put the occ chunks last, so
the DVE mask chain could not start until ~24us. Now:
  - table chunks stream on the sync ring; occ chunks + small weights go on
    the scalar ring, landing in the first few us.
  - per-(strip, h) output tiles; h0 stores dispatched on sync, h1 on
    scalar, so stores overlap the tail of the load stream.
  - TT emission order matches ACT completion order (strip-major).
HBM/core: tbl 3.28 MB bf16 + occ 1.64 MB i8 + out 3.28 MB bf16 = 8.2 MB.
"""

import numpy as np
import ml_dtypes

import concourse.bass as bass
import concourse.tile as tile
from concourse import bacc, mybir
from concourse.bass_utils import run_bass_kernel_spmd

B = 256
L = 512
V = 50257
DW = 256
DA = 256
ALPHA = 0.2

NCORES = 8
VS = 6400          # vocab span per core
SW = 1280          # strip width
NS = VS // SW      # 5 strips
GA = 2 * SW        # vocab-group A: strips 0-1  -> cols [0, 2560)
GB = VS - GA       # vocab-group B: strips 2-4  -> cols [2560, 6400)

BF16 = ml_dtypes.bfloat16

_CACHE = {}


def _build():
    if "nc" in _CACHE:
        return _CACHE["nc"]
    f32 = mybir.dt.float32
    bf16 = mybir.dt.bfloat16
    i8 = mybir.dt.int8

    nc = bacc.Bacc("TRN2", target_bir_lowering=False, debug=False)
    tbl = nc.declare_dram_parameter("tbl", [128, 2 * VS], bf16, isOutput=False)
    occ = nc.declare_dram_parameter("occ", [128, 2 * VS], i8, isOutput=False)
    awb = nc.declare_dram_parameter("awb", [128, 2 * 128], bf16, isOutput=False)
    aa = nc.declare_dram_parameter("aa", [128, DA], f32, isOutput=False)
    attr = nc.declare_dram_parameter("attr", [128, 2 * DA], f32, isOutput=False)
    out = nc.declare_dram_parameter("out", [128, 2 * VS], bf16, isOutput=True)

    with tile.TileContext(nc) as tc:
        with (
            tc.tile_pool(name="sb", bufs=1) as sb,
            tc.tile_pool(name="pst", bufs=2, space="PSUM") as pst,
        ):
            # ---- table chunks on the sync ring (the big load stream) ----
            tA = [sb.tile([128, GA], bf16, tag=f"tA{dh}", name=f"tA{dh}") for dh in range(2)]
            tB = [sb.tile([128, GB], bf16, tag=f"tB{dh}", name=f"tB{dh}") for dh in range(2)]
            for dh in range(2):
                nc.sync.dma_start(tA[dh][:], tbl.ap()[:, dh * VS : dh * VS + GA])
            for dh in range(2):
                nc.sync.dma_start(tB[dh][:], tbl.ap()[:, dh * VS + GA : (dh + 1) * VS])

            # ---- small weights + occ chunks on the scalar ring ----
            awb_t = sb.tile([128, 2 * 128], bf16, tag="awb")
            nc.scalar.dma_start(awb_t[:], awb.ap())
            aa_t = sb.tile([128, DA], f32, tag="aa")
            nc.scalar.dma_start(aa_t[:], aa.ap())
            at = sb.tile([128, 2 * DA], f32, tag="attr")
            nc.scalar.dma_start(at[:], attr.ap())
            # occ[h][grp]
            ocg = [[None, None], [None, None]]
            for h in range(2):
                for grp, (c0, c1) in enumerate(((0, GA), (GA, VS))):
                    t_ = sb.tile([128, c1 - c0], i8, tag=f"oc{h}{grp}", name=f"oc{h}{grp}")
                    nc.scalar.dma_start(t_[:], occ.ap()[:, h * VS + c0 : h * VS + c1])
                    ocg[h][grp] = t_

            # ---- s = attr_emb @ a_a  (s_sb[:, h] holds b = h*128 + p) ----
            s_sb = sb.tile([128, 2], f32, tag="s")
            for h in range(2):
                pa = sb.tile([128, DA], f32, tag=f"pa{h}")
                nc.vector.tensor_tensor(
                    out=pa[:],
                    in0=at[:, h * DA : (h + 1) * DA],
                    in1=aa_t[:],
                    op=mybir.AluOpType.mult,
                )
                nc.vector.tensor_reduce(
                    out=s_sb[:, h : h + 1],
                    in_=pa[:],
                    axis=mybir.AxisListType.X,
                    op=mybir.AluOpType.add,
                )

            # ---- per strip: matmul + both ACT passes (drains PSUM fast) ----
            o1s = {}
            for si in range(NS):
                grp = 0 if si < 2 else 1
                tt = tA if grp == 0 else tB
                base = 0 if grp == 0 else GA
                off = si * SW - base
                pt = pst.tile([128, SW], f32, tag="pt")
                for dh in range(2):
                    for n0, n1 in ((0, 512), (512, 1024), (1024, SW)):
                        nc.tensor.matmul(
                            pt[:, n0:n1],
                            lhsT=awb_t[:, dh * 128 : (dh + 1) * 128],
                            rhs=tt[dh][:, off + n0 : off + n1],
                            start=(dh == 0),
                            stop=(dh == 1),
                        )
                for h in range(2):
                    o1 = sb.tile([128, SW], bf16, tag=f"o1_{si}_{h}", name=f"o1_{si}_{h}")
                    nc.scalar.activation(
                        o1[:],
                        pt[:],
                        mybir.ActivationFunctionType.Prelu,
                        bias=s_sb[:, h : h + 1],
                        scale=1.0,
                        alpha=ALPHA,
                    )
                    o1s[(si, h)] = (grp, off, o1)

            # ---- mask + per-(strip,h) store; h0 on sync, h1 on scalar ----
            for si in range(NS):
                for h in range(2):
                    grp, off, o1 = o1s[(si, h)]
                    o = sb.tile([128, SW], bf16, tag=f"o_{si}_{h}", name=f"o_{si}_{h}")
                    nc.vector.tensor_tensor(
                        out=o[:],
                        in0=o1[:],
                        in1=ocg[h][grp][:, off : off + SW],
                        op=mybir.AluOpType.mult,
                    )
                    eng = nc.sync if h == 0 else nc.scalar
                    eng.dma_start(
                        out.ap()[:, h * VS + si * SW : h * VS + (si + 1) * SW],
                        o[:],
                    )

    nc.compile()
    _CACHE["nc"] = nc
    return nc


def _pmaj(x):
    """[256, N] -> partition-major [128, 2*N] (halves along columns)."""
    n = x.shape[1]
    return np.ascontiguousarray(
        x.reshape(2, 128, n).transpose(1, 0, 2).reshape(128, 2 * n)
    )


def _prep_inputs(words, word_emb_table, attr_emb, a):
    words = np.ascontiguousarray(words).astype(np.int64)
    wet = np.ascontiguousarray(word_emb_table, dtype=np.float32)
    attr = np.ascontiguousarray(attr_emb, dtype=np.float32)
    a = np.ascontiguousarray(a, dtype=np.float32).reshape(-1)

    # awb_dev[p, dh*128 + m] = a[dh*128 + p]
    A = a[:DW].astype(BF16).reshape(2, 128)
    awb_dev = np.ascontiguousarray(
        np.repeat(A.T[:, :, None], 128, axis=2).reshape(128, 2 * 128)
    )
    aa_rep = np.ascontiguousarray(np.broadcast_to(a[DW:][None, :], (128, DA)))
    attr_dev = _pmaj(attr)

    tblpad = np.zeros((NCORES * VS, DW), dtype=np.float32)
    tblpad[:V] = wet
    tbl_bf = tblpad.astype(BF16)

    occ_full = np.zeros((B, NCORES * VS), dtype=np.int8)
    rows = np.repeat(np.arange(B), L)
    occ_full[rows, words.reshape(-1)] = 1

    in_maps = []
    for i in range(NCORES):
        blk = tbl_bf[i * VS : (i + 1) * VS, :]          # [VS, 256]
        tbl_dev = _pmaj(np.ascontiguousarray(blk.T))    # [128, 2*VS]
        occ_dev = _pmaj(occ_full[:, i * VS : (i + 1) * VS])
        in_maps.append(
            {
                "tbl": tbl_dev,
                "occ": occ_dev,
                "awb": awb_dev,
                "aa": aa_rep,
                "attr": attr_dev,
            }
        )
    return in_maps


def kernel(words, word_emb_table, attr_emb, a, _trace=False, **_kw):
    nc = _build()
    in_maps = _prep_inputs(words, word_emb_table, attr_emb, a)
    res = run_bass_kernel_spmd(nc, in_maps, list(range(NCORES)), trace=_trace)
    parts = []
    for i in range(NCORES):
        o = res.results[i]["out"]                       # [128, 2*VS] bf16
        parts.append(o.reshape(128, 2, VS).transpose(1, 0, 2).reshape(B, VS))
    out = np.ascontiguousarray(
        np.concatenate(parts, axis=1)[:, :V].astype(np.float32)
    )
    if _trace:
        return out, res
    return out


# revision 7
# speedup vs baseline: 1.1410x; 1.0395x over previous
"""Trainium2 Bass kernel v6 for nn_AttentionLayer.

Math (per core, vocab-sharded): out[b, v'] = occ[b, v'] * leaky_relu(t[v'] + s[b])
with t = table_shard^T a_w (PE, bf16), s = attr_emb @ a_a (DVE, f32).

v5 vs v4 (46.4us): v4's single sync-ring FIFO put the occ chunks last, so
the DVE mask chain could not start until ~24us. Now:
  - table chunks stream on the sync ring; occ chunks + small weights go on
    the scalar ring, landing in the first few us.
  - per-(strip, h) output tiles; h0 stores dispatched on sync, h1 on
    scalar, so stores overlap the tail of the load stream.
  - TT emission order matches ACT completion order (strip-major).
HBM/core: tbl 3.28 MB bf16 + occ 1.64 MB i8 + out 3.28 MB bf16 = 8.2 MB.
"""

import numpy as np
import ml_dtypes

import concourse.bass as bass
import concourse.tile as tile
from concourse import bacc, mybir
from concourse.bass_utils import run_bass_kernel_spmd

B = 256
L = 512
V = 50257
DW = 256
DA = 256
ALPHA = 0.2

NCORES = 8
VS = 6400          # vocab span per core
SW = 1280          # strip width
NS = VS // SW      # 5 strips
GA = 2 * SW        # vocab-group A: strips 0-1  -> cols [0, 2560)
GB = VS - GA       # vocab-group B: strips 2-4  -> cols [2560, 6400)

BF16 = ml_dtypes.bfloat16

_CACHE = {}


def _build():
    if "nc" in _CACHE:
        return _CACHE["nc"]
    f32 = mybir.dt.float32
    bf16 = mybir.dt.bfloat16
    i8 = mybir.dt.int8

    nc = bacc.Bacc("TRN2", target_bir_lowering=False, debug=False)
    tbl = nc.declare_dram_parameter("tbl", [128, 2 * VS], bf16, isOutput=False)
    occ = nc.declare_dram_parameter("occ", [128, 2 * VS], i8, isOutput=False)
    awb = nc.declare_dram_parameter("awb", [128, 2 * 128], bf16, isOutput=False)
    aa = nc.declare_dram_parameter("aa", [128, DA], f32, isOutput=False)
    attr = nc.declare_dram_parameter("attr", [128, 2 * DA], f32, isOutput=False)
    out = nc.declare_dram_parameter("out", [128, 2 * VS], bf16, isOutput=True)

    with tile.TileContext(nc) as tc:
        with (
            tc.tile_pool(name="sb", bufs=1) as sb,
            tc.tile_pool(name="pst", bufs=2, space="PSUM") as pst,
        ):
            # ---- tiny weight loads first so s / matmuls start early ----
            aa_t = sb.tile([128, DA], f32, tag="aa")
            nc.sync.dma_start(aa_t[:], aa.ap())
            at = sb.tile([128, 2 * DA], f32, tag="attr")
            nc.sync.dma_start(at[:], attr.ap())
            awb_t = sb.tile([128, 2 * 128], bf16, tag="awb")
            nc.scalar.dma_start(awb_t[:], awb.ap())

            # ---- table: strip x dh chunks on the sync ring ----
            ts = {}
            for si in range(NS):
                for dh in range(2):
                    t_ = sb.tile([128, SW], bf16, tag=f"t{si}{dh}", name=f"t{si}{dh}")
                    nc.sync.dma_start(
                        t_[:],
                        tbl.ap()[:, dh * VS + si * SW : dh * VS + (si + 1) * SW],
                    )
                    ts[(si, dh)] = t_

            # ---- occ chunks on the scalar ring, A-group first ----
            ocg = [[None, None], [None, None]]
            for grp, (c0, c1) in enumerate(((0, GA), (GA, VS))):
                for h in range(2):
                    t_ = sb.tile([128, c1 - c0], i8, tag=f"oc{h}{grp}", name=f"oc{h}{grp}")
                    nc.scalar.dma_start(t_[:], occ.ap()[:, h * VS + c0 : h * VS + c1])
                    ocg[h][grp] = t_

            # ---- s = attr_emb @ a_a  (s_sb[:, h] holds b = h*128 + p) ----
            s_sb = sb.tile([128, 2], f32, tag="s")
            for h in range(2):
                pa = sb.tile([128, DA], f32, tag=f"pa{h}")
                nc.vector.tensor_tensor(
                    out=pa[:],
                    in0=at[:, h * DA : (h + 1) * DA],
                    in1=aa_t[:],
                    op=mybir.AluOpType.mult,
                )
                nc.vector.tensor_reduce(
                    out=s_sb[:, h : h + 1],
                    in_=pa[:],
                    axis=mybir.AxisListType.X,
                    op=mybir.AluOpType.add,
                )

            # ---- per strip: matmul + both ACT passes (drains PSUM fast) ----
            o1s = {}
            for si in range(NS):
                grp = 0 if si < 2 else 1
                base = 0 if grp == 0 else GA
                off = si * SW - base
                pt = pst.tile([128, SW], f32, tag="pt")
                for dh in range(2):
                    for n0, n1 in ((0, 512), (512, 1024), (1024, SW)):
                        nc.tensor.matmul(
                            pt[:, n0:n1],
                            lhsT=awb_t[:, dh * 128 : (dh + 1) * 128],
                            rhs=ts[(si, dh)][:, n0:n1],
                            start=(dh == 0),
                            stop=(dh == 1),
                        )
                for h in range(2):
                    o1 = sb.tile([128, SW], bf16, tag=f"o1_{si}_{h}", name=f"o1_{si}_{h}")
                    nc.scalar.activation(
                        o1[:],
                        pt[:],
                        mybir.ActivationFunctionType.Prelu,
                        bias=s_sb[:, h : h + 1],
                        scale=1.0,
                        alpha=ALPHA,
                    )
                    o1s[(si, h)] = (grp, off, o1)

            # ---- mask + per-(strip,h) store; h0 on sync, h1 on scalar ----
            for si in range(NS):
                for h in range(2):
                    grp, off, o1 = o1s[(si, h)]
                    o = sb.tile([128, SW], bf16, tag=f"o_{si}_{h}", name=f"o_{si}_{h}")
                    nc.vector.tensor_tensor(
                        out=o[:],
                        in0=o1[:],
                        in1=ocg[h][grp][:, off : off + SW],
                        op=mybir.AluOpType.mult,
                    )
                    eng = nc.sync if h == 0 else nc.scalar
                    eng.dma_start(
                        out.ap()[:, h * VS + si * SW : h * VS + (si + 1) * SW],
                        o[:],
                    )

    nc.compile()
    _CACHE["nc"] = nc
    return nc


def _pmaj(x):
    """[256, N] -> partition-major [128, 2*N] (halves along columns)."""
    n = x.shape[1]
    return np.ascontiguousarray(
        x.reshape(2, 128, n).transpose(1, 0, 2).reshape(128, 2 * n)
    )


def _prep_inputs(words, word_emb_table, attr_emb, a):
    words = np.ascontiguousarray(words).astype(np.int64)
    wet = np.ascontiguousarray(word_emb_table, dtype=np.float32)
    attr = np.ascontiguousarray(attr_emb, dtype=np.float32)
    a = np.ascontiguousarray(a, dtype=np.float32).reshape(-1)

    # awb_dev[p, dh*128 + m] = a[dh*128 + p]
    A = a[:DW].astype(BF16).reshape(2, 128)
    awb_dev = np.ascontiguousarray(
        np.repeat(A.T[:, :, None], 128, axis=2).reshape(128, 2 * 128)
    )
    aa_rep = np.ascontiguousarray(np.broadcast_to(a[DW:][None, :], (128, DA)))
    attr_dev = _pmaj(attr)

    tblpad = np.zeros((NCORES * VS, DW), dtype=np.float32)
    tblpad[:V] = wet
    tbl_bf = tblpad.astype(BF16)

    occ_full = np.zeros((B, NCORES * VS), dtype=np.int8)
    rows = np.repeat(np.arange(B), L)
    occ_full[rows, words.reshape(-1)] = 1

    in_maps = []
    for i in range(NCORES):
        blk = tbl_bf[i * VS : (i + 1) * VS, :]          # [VS, 256]
        tbl_dev = _pmaj(np.ascontiguousarray(blk.T))    # [128, 2*VS]
        occ_dev = _pmaj(occ_full[:, i * VS : (i + 1) * VS])
        in_maps.append(
            {
                "tbl": tbl_dev,
                "occ": occ_dev,
                "awb": awb_dev,
                "aa": aa_rep,
                "attr": attr_dev,
            }
        )
    return in_maps


def kernel(words, word_emb_table, attr_emb, a, _trace=False, **_kw):
    nc = _build()
    in_maps = _prep_inputs(words, word_emb_table, attr_emb, a)
    res = run_bass_kernel_spmd(nc, in_maps, list(range(NCORES)), trace=_trace)
    parts = []
    for i in range(NCORES):
        o = res.results[i]["out"]                       # [128, 2*VS] bf16
        parts.append(o.reshape(128, 2, VS).transpose(1, 0, 2).reshape(B, VS))
    out = np.ascontiguousarray(
        np.concatenate(parts, axis=1)[:, :V].astype(np.float32)
    )
    if _trace:
        return out, res
    return out
